# revision 30
# baseline (speedup 1.0000x reference)
"""Trainium2 Bass kernel: BN(eval) -> sign -> Conv1d(K=7,pad=3) -> alpha -> PReLU -> MaxPool2.

Strategy (hardcoded for B=64, CIN=64, L=4096, COUT=128, K=7):
  - Data-parallel over batch: 8 samples per NeuronCore x 8 cores; no
    cross-core communication.
  - Host folds BN into per-channel (scale, bias) and alpha into the conv
    weights (bf16); per-channel PReLU slope rides in as an SBUF vector.
  - A PAIR of samples shares one [128, L+8] bf16 "sign" tile: rows 0-63 =
    sample A, rows 64-127 = sample B (one chunked ScalarE Sign activation
    per input-DMA chunk so matmuls start early).
  - Conv = 7 PSUM-accumulated K=64 bf16 matmuls per 512-col tile; sample
    A's matmuls run on PE row-group 0-1 and B's on row-group 2-3
    concurrently (weights duplicated into both halves of the array), which
    measures ~94% of bf16 peak. A 30-matmul warmup flips the HAM clock
    gate to 8/8 before real work, and a dummy activation hoists the ACT
    table load to kernel start.
  - MaxPool(2) straight out of PSUM via DVE tensor_reduce(max) on
    [128, 256, 2] views; PReLU applied AFTER pooling (they commute) via
    the native ScalarE Prelu activation on bf16 halves, DMA'd out as bf16
    and widened to fp32 on the host.
  - Walrus in this toolchain accepts only one sync-wait per instruction,
    so the Tile-scheduled BIR is post-processed: multi-wait sync_info
    lists become single-wait EventSemaphore instructions (see
    _split_sync_waits_json).
"""

import json
import sys

for _p in ("/opt/trn_rl_repo", "/root/.axon_site/_ro/trn_rl_repo"):
    if _p not in sys.path:
        sys.path.append(_p)

import numpy as np
import ml_dtypes

import concourse.bass as bass
import concourse.tile as tile
from concourse import mybir
from concourse.bass_utils import run_bass_kernel_spmd

B, CIN, L, COUT, K = 64, 64, 4096, 128, 7
PAD = 3
BN_EPS = 1e-5
N_CORES = 8
BPC = B // N_CORES  # samples per core
LOUT = L // 2       # 2048 pooled length
NT = L // 512       # 8 output tiles of 512 cols

_CACHE: dict = {}


def build_program(use_act_prelu: bool = True) -> "bass.Bass":
    nc = bass.Bass(trn_type="TRN2")
    I8 = nc.dram_tensor("I8", [BPC, CIN, L], mybir.dt.float32, kind="ExternalInput")
    W = nc.dram_tensor("W", [128, K * 128], mybir.dt.bfloat16, kind="ExternalInput")
    SBp = nc.dram_tensor("SBp", [128, 4], mybir.dt.float32, kind="ExternalInput")
    O8 = nc.dram_tensor("O8", [BPC, COUT, LOUT], mybir.dt.bfloat16, kind="ExternalOutput")

    iflat = I8.ap().flatten_outer_dims()  # [BPC*64, 4096]
    oflat = O8.ap().flatten_outer_dims()  # [BPC*128, 2048]

    AF = mybir.ActivationFunctionType
    SGW = L + 8  # sg width: cols 0-2 zero pad, 3..L+2 data, L+3.. zero
    NHALF = NT // 2  # 4 l-tiles per half (A half + B half = 8 PSUM banks)
    with tile.TileContext(nc) as tc:
        with (
            tc.tile_pool(name="consts", bufs=1) as consts,
            tc.tile_pool(name="ipair", bufs=2) as ipool,
            tc.tile_pool(name="sgn", bufs=2) as spool,
            tc.tile_pool(name="pooled", bufs=2) as plpool,
            tc.tile_pool(name="outp", bufs=3) as opool,
            tc.tile_pool(name="ps", bufs=8, space="PSUM") as pspool,
        ):
            w_sb = consts.tile([128, K * 128], mybir.dt.bfloat16)
            nc.scalar.dma_start(w_sb[:], W.ap()[:])
            sb_sb = consts.tile([128, 4], mybir.dt.float32)
            nc.scalar.dma_start(sb_sb[:], SBp.ap()[:])
            # dummy activation: hoists the ACT table load to kernel start so
            # the first real Sign doesn't pay the ~1.3us table fetch later
            dummy = consts.tile([128, 4], mybir.dt.float32)
            nc.scalar.activation(
                dummy[:], sb_sb[:], mybir.ActivationFunctionType.Sign
            )
            # PE warmup while the first input chunks stream in: ~30 tiny
            # matmuls on the weight tile flip the HAM clock gate to 8/8
            warm = pspool.tile([128, 512], mybir.dt.float32, name="warm", tag="psb")
            for _ in range(30):
                nc.tensor.matmul(
                    warm[:, 0:64], w_sb[0:64, 0:128], w_sb[0:64, 0:64],
                    start=True, stop=True,
                )
            sgn_scale = sb_sb[:, 0:1]
            sgn_bias = sb_sb[:, 1:2]
            slope = sb_sb[:, 3:4]  # a

            NCHUNK = 8
            CW = L // NCHUNK
            for t in range(BPC // 2):
                # chunked input DMA + sign so the first matmuls start early
                ip = ipool.tile([128, L], mybir.dt.float32)
                sg = spool.tile([128, SGW], mybir.dt.bfloat16)
                nc.gpsimd.memset(sg[:, 0:3], 0.0)
                nc.gpsimd.memset(sg[:, L + 3 : SGW], 0.0)
                for c in range(NCHUNK):
                    nc.sync.dma_start(
                        ip[:, CW * c : CW * (c + 1)],
                        iflat[128 * t : 128 * (t + 1), CW * c : CW * (c + 1)],
                    )
                    nc.scalar.activation(
                        sg[:, 3 + CW * c : 3 + CW * (c + 1)],
                        ip[:, CW * c : CW * (c + 1)],
                        AF.Sign, bias=sgn_bias, scale=sgn_scale,
                    )

                pla = plpool.tile([128, LOUT], mybir.dt.bfloat16, name="pla", tag="pla")
                plb = plpool.tile([128, LOUT], mybir.dt.bfloat16, name="plb", tag="plb")
                for it in range(NT):
                    psa = pspool.tile([128, 512], mybir.dt.float32, name="psa", tag="psb")
                    psb = pspool.tile([128, 512], mybir.dt.float32, name="psb", tag="psb")
                    for k in range(K):
                        c0 = 512 * it + k
                        nc.tensor.matmul(
                            psa[:], w_sb[0:64, 128 * k : 128 * (k + 1)],
                            sg[0:64, c0 : c0 + 512],
                            start=(k == 0), stop=(k == K - 1),
                        )
                        nc.tensor.matmul(
                            psb[:], w_sb[64:128, 128 * k : 128 * (k + 1)],
                            sg[64:128, c0 : c0 + 512],
                            start=(k == 0), stop=(k == K - 1),
                        )
                    o0 = 256 * it
                    nc.vector.tensor_reduce(
                        pla[:, o0 : o0 + 256],
                        psa[:].rearrange("p (n two) -> p n two", two=2),
                        mybir.AxisListType.X,
                        mybir.AluOpType.max,
                    )
                    nc.vector.tensor_reduce(
                        plb[:, o0 : o0 + 256],
                        psb[:].rearrange("p (n two) -> p n two", two=2),
                        mybir.AxisListType.X,
                        mybir.AluOpType.max,
                    )
                    # flush policy: halves normally (coarse out-DMAs never
                    # queue ahead of the next pair's input chunks); for the
                    # LAST pair's second half flush per-tile quarters so the
                    # tail drains right behind the final matmuls
                    last_pair = t == BPC // 2 - 1
                    if last_pair and it >= NT // 2:
                        spans = [(256 * it, 256)]
                    elif it == NT // 2 - 1 or (not last_pair and it == NT - 1):
                        spans = [(LOUT // 2 * (it // (NT // 2)), LOUT // 2)]
                    else:
                        spans = []
                    for s0, sw in spans:
                        # prelu on the pooled span (prelu commutes with max)
                        for h, pl in ((0, pla), (1, plb)):
                            b = 2 * t + h
                            o = opool.tile(
                                [128, sw], mybir.dt.bfloat16, name="o", tag="o"
                            )
                            if use_act_prelu:
                                nc.scalar.activation(
                                    o[:], pl[:, s0 : s0 + sw], AF.Prelu,
                                    alpha=slope,
                                )
                            else:
                                nc.vector.scalar_tensor_tensor(
                                    o[:], pl[:, s0 : s0 + sw], slope,
                                    pl[:, s0 : s0 + sw],
                                    mybir.AluOpType.mult, mybir.AluOpType.max,
                                )
                            nc.sync.dma_start(
                                oflat[128 * b : 128 * (b + 1), s0 : s0 + sw],
                                o[:],
                            )
    return nc


def _split_sync_waits_json(bir: bytes) -> bytes:
    """Walrus in this toolchain accepts at most one sync-wait per instruction.
    Hoist multi-wait sync_info lists into preceding single-wait EventSemaphore
    instructions on the same engine queue (the same form engine.wait_ge()
    lowers to), preserving program order and on_update placement."""
    j = json.loads(bir)
    n_split = 0
    for fn in j.get("functions", []):
        for blk in fn.get("blocks", []):
            ins_list = blk.get("instructions")
            if not ins_list:
                continue
            out = []
            for ins in ins_list:
                si = ins.get("sync_info")
                waits = si.get("on_wait") if si else None
                if waits and len(waits) > 1:
                    for i, w in enumerate(waits):
                        out.append(
                            {
                                "debug": ins.get("debug", 0),
                                "engine": ins["engine"],
                                "ins": [],
                                "outs": [],
                                "name": f"{ins['name']}-antw{i}",
                                "opcode": "EventSemaphore",
                                "sync_info": {"on_update": [], "on_wait": [w]},
                            }
                        )
                    si["on_wait"] = []
                    n_split += 1
                out.append(ins)
            blk["instructions"] = out
    return json.dumps(j).encode()


def get_program() -> "bass.Bass":
    if "nc" not in _CACHE:
        nc = build_program()
        orig = nc.to_json_bytes
        nc.to_json_bytes = lambda: _split_sync_waits_json(orig())
        _CACHE["nc"] = nc
    return _CACHE["nc"]


def prep_inputs(I, bn_gamma, bn_beta, bn_mean, bn_var, conv_w, alpha, prelu_w):
    """Host-side folding: BN -> (scale, bias); alpha -> weights; per-k lhsT
    blocks duplicated into both PE array halves."""
    f32 = np.float32
    gamma = np.asarray(bn_gamma, f32)
    beta = np.asarray(bn_beta, f32)
    mean = np.asarray(bn_mean, f32)
    var = np.asarray(bn_var, f32)
    s = gamma / np.sqrt(var + f32(BN_EPS))        # [CIN]
    t = beta - mean * s                            # [CIN]

    w = np.asarray(conv_w, f32) * np.asarray(alpha, f32)[:, None, None]  # [COUT, CIN, K]
    Wb = np.zeros((128, K * 128), np.float32)
    for k in range(K):
        Wb[0:64, 128 * k : 128 * k + 128] = w[:, :, k].T
        Wb[64:128, 128 * k : 128 * k + 128] = w[:, :, k].T
    Wb = Wb.astype(ml_dtypes.bfloat16)

    a = f32(np.asarray(prelu_w, f32).reshape(-1)[0])
    sbp = np.zeros((128, 4), f32)
    sbp[0:64, 0] = s
    sbp[64:128, 0] = s
    sbp[0:64, 1] = t
    sbp[64:128, 1] = t
    sbp[:, 2] = f32(1.0) - a
    sbp[:, 3] = a
    return Wb, sbp


def kernel(I, bn_gamma, bn_beta, bn_mean, bn_var, conv_w, alpha, prelu_w):
    I = np.ascontiguousarray(np.asarray(I, np.float32))
    assert I.shape == (B, CIN, L), I.shape
    Wb, sbp = prep_inputs(I, bn_gamma, bn_beta, bn_mean, bn_var, conv_w, alpha, prelu_w)

    nc = get_program()
    in_maps = [
        {"I8": I[BPC * c : BPC * (c + 1)], "W": Wb, "SBp": sbp} for c in range(N_CORES)
    ]
    res = run_bass_kernel_spmd(nc, in_maps, core_ids=list(range(N_CORES)))
    out = np.concatenate(
        [np.asarray(res.results[c]["O8"]) for c in range(N_CORES)], axis=0
    )
    return np.ascontiguousarray(out.astype(np.float32))



# revision 31
# speedup vs baseline: 1.0284x; 1.0284x over previous
"""Trainium2 Bass kernel: BN(eval) -> sign -> Conv1d(K=7,pad=3) -> alpha -> PReLU -> MaxPool2.

Strategy (hardcoded for B=64, CIN=64, L=4096, COUT=128, K=7):
  - Data-parallel over batch: 8 samples per NeuronCore x 8 cores; no
    cross-core communication.
  - Host folds BN into per-channel (scale, bias) and alpha into the conv
    weights (bf16); per-channel PReLU slope rides in as an SBUF vector.
  - A PAIR of samples shares one [128, L+8] bf16 "sign" tile: rows 0-63 =
    sample A, rows 64-127 = sample B (one chunked ScalarE Sign activation
    per input-DMA chunk so matmuls start early).
  - Conv = 7 PSUM-accumulated K=64 bf16 matmuls per 512-col tile; sample
    A's matmuls run on PE row-group 0-1 and B's on row-group 2-3
    concurrently (weights duplicated into both halves of the array), which
    measures ~94% of bf16 peak. A 30-matmul warmup flips the HAM clock
    gate to 8/8 before real work, and a dummy activation hoists the ACT
    table load to kernel start.
  - MaxPool(2) straight out of PSUM via DVE tensor_reduce(max) on
    [128, 256, 2] views; PReLU applied AFTER pooling (they commute) via
    the native ScalarE Prelu activation on bf16 halves, DMA'd out as bf16
    and widened to fp32 on the host.
  - Walrus in this toolchain accepts only one sync-wait per instruction,
    so the Tile-scheduled BIR is post-processed: multi-wait sync_info
    lists become single-wait EventSemaphore instructions (see
    _split_sync_waits_json).
"""

import json
import sys

for _p in ("/opt/trn_rl_repo", "/root/.axon_site/_ro/trn_rl_repo"):
    if _p not in sys.path:
        sys.path.append(_p)

import numpy as np
import ml_dtypes

import concourse.bass as bass
import concourse.tile as tile
from concourse import mybir
from concourse.bass_utils import run_bass_kernel_spmd

B, CIN, L, COUT, K = 64, 64, 4096, 128, 7
PAD = 3
BN_EPS = 1e-5
N_CORES = 8
BPC = B // N_CORES  # samples per core
LOUT = L // 2       # 2048 pooled length
NT = L // 512       # 8 output tiles of 512 cols

_CACHE: dict = {}


def build_program(use_act_prelu: bool = True) -> "bass.Bass":
    nc = bass.Bass(trn_type="TRN2")
    I8 = nc.dram_tensor("I8", [BPC, CIN, L], mybir.dt.float32, kind="ExternalInput")
    W = nc.dram_tensor("W", [128, K * 128], mybir.dt.bfloat16, kind="ExternalInput")
    SBp = nc.dram_tensor("SBp", [128, 4], mybir.dt.float32, kind="ExternalInput")
    O8 = nc.dram_tensor("O8", [BPC, COUT, LOUT], mybir.dt.bfloat16, kind="ExternalOutput")

    iflat = I8.ap().flatten_outer_dims()  # [BPC*64, 4096]
    oflat = O8.ap().flatten_outer_dims()  # [BPC*128, 2048]

    AF = mybir.ActivationFunctionType
    SGW = L + 8  # sg width: cols 0-2 zero pad, 3..L+2 data, L+3.. zero
    NHALF = NT // 2  # 4 l-tiles per half (A half + B half = 8 PSUM banks)
    with tile.TileContext(nc) as tc:
        with (
            tc.tile_pool(name="consts", bufs=1) as consts,
            tc.tile_pool(name="ipair", bufs=2) as ipool,
            tc.tile_pool(name="sgn", bufs=2) as spool,
            tc.tile_pool(name="pooled", bufs=2) as plpool,
            tc.tile_pool(name="outp", bufs=3) as opool,
            tc.tile_pool(name="ps", bufs=8, space="PSUM") as pspool,
        ):
            w_sb = consts.tile([128, K * 128], mybir.dt.bfloat16)
            nc.scalar.dma_start(w_sb[:], W.ap()[:])
            sb_sb = consts.tile([128, 4], mybir.dt.float32)
            nc.scalar.dma_start(sb_sb[:], SBp.ap()[:])
            # dummy activation: hoists the ACT table load to kernel start so
            # the first real Sign doesn't pay the ~1.3us table fetch later
            dummy = consts.tile([128, 4], mybir.dt.float32)
            nc.scalar.activation(
                dummy[:], sb_sb[:], mybir.ActivationFunctionType.Sign
            )
            # PE warmup while the first input chunks stream in: ~30 tiny
            # matmuls on the weight tile flip the HAM clock gate to 8/8
            warm = pspool.tile([128, 512], mybir.dt.float32, name="warm", tag="psb")
            for _ in range(30):
                nc.tensor.matmul(
                    warm[:, 0:64], w_sb[0:64, 0:128], w_sb[0:64, 0:64],
                    start=True, stop=True,
                )
            sgn_scale = sb_sb[:, 0:1]
            sgn_bias = sb_sb[:, 1:2]
            slope = sb_sb[:, 3:4]  # a

            NCHUNK = 8
            CW = L // NCHUNK
            for t in range(BPC // 2):
                # chunked input DMA + sign so the first matmuls start early
                ip = ipool.tile([128, L], mybir.dt.float32)
                sg = spool.tile([128, SGW], mybir.dt.bfloat16)
                nc.gpsimd.memset(sg[:, 0:3], 0.0)
                nc.gpsimd.memset(sg[:, L + 3 : SGW], 0.0)
                for c in range(NCHUNK):
                    nc.sync.dma_start(
                        ip[:, CW * c : CW * (c + 1)],
                        iflat[128 * t : 128 * (t + 1), CW * c : CW * (c + 1)],
                    )
                    nc.scalar.activation(
                        sg[:, 3 + CW * c : 3 + CW * (c + 1)],
                        ip[:, CW * c : CW * (c + 1)],
                        AF.Sign, bias=sgn_bias, scale=sgn_scale,
                    )

                pla = plpool.tile([128, LOUT], mybir.dt.bfloat16, name="pla", tag="pla")
                plb = plpool.tile([128, LOUT], mybir.dt.bfloat16, name="plb", tag="plb")
                for it in range(NT):
                    psa = pspool.tile([128, 512], mybir.dt.float32, name="psa", tag="psb")
                    psb = pspool.tile([128, 512], mybir.dt.float32, name="psb", tag="psb")
                    for k in range(K):
                        c0 = 512 * it + k
                        nc.tensor.matmul(
                            psa[:], w_sb[0:64, 128 * k : 128 * (k + 1)],
                            sg[0:64, c0 : c0 + 512],
                            start=(k == 0), stop=(k == K - 1),
                        )
                        nc.tensor.matmul(
                            psb[:], w_sb[64:128, 128 * k : 128 * (k + 1)],
                            sg[64:128, c0 : c0 + 512],
                            start=(k == 0), stop=(k == K - 1),
                        )
                    o0 = 256 * it
                    nc.vector.tensor_reduce(
                        pla[:, o0 : o0 + 256],
                        psa[:].rearrange("p (n two) -> p n two", two=2),
                        mybir.AxisListType.X,
                        mybir.AluOpType.max,
                    )
                    nc.vector.tensor_reduce(
                        plb[:, o0 : o0 + 256],
                        psb[:].rearrange("p (n two) -> p n two", two=2),
                        mybir.AxisListType.X,
                        mybir.AluOpType.max,
                    )
                    # flush pooled halves: coarse out-DMAs so they never
                    # queue ahead of the next pair's input chunks
                    if it == NT // 2 - 1 or it == NT - 1:
                        spans = [(LOUT // 2 * (it // (NT // 2)), LOUT // 2)]
                    else:
                        spans = []
                    for s0, sw in spans:
                        # prelu on the pooled span (prelu commutes with max)
                        for h, pl in ((0, pla), (1, plb)):
                            b = 2 * t + h
                            o = opool.tile(
                                [128, sw], mybir.dt.bfloat16, name="o", tag="o"
                            )
                            if use_act_prelu:
                                nc.scalar.activation(
                                    o[:], pl[:, s0 : s0 + sw], AF.Prelu,
                                    alpha=slope,
                                )
                            else:
                                nc.vector.scalar_tensor_tensor(
                                    o[:], pl[:, s0 : s0 + sw], slope,
                                    pl[:, s0 : s0 + sw],
                                    mybir.AluOpType.mult, mybir.AluOpType.max,
                                )
                            nc.sync.dma_start(
                                oflat[128 * b : 128 * (b + 1), s0 : s0 + sw],
                                o[:],
                            )
    return nc


def _split_sync_waits_json(bir: bytes) -> bytes:
    """Walrus in this toolchain accepts at most one sync-wait per instruction.
    Hoist multi-wait sync_info lists into preceding single-wait EventSemaphore
    instructions on the same engine queue (the same form engine.wait_ge()
    lowers to), preserving program order and on_update placement."""
    j = json.loads(bir)
    n_split = 0
    for fn in j.get("functions", []):
        for blk in fn.get("blocks", []):
            ins_list = blk.get("instructions")
            if not ins_list:
                continue
            out = []
            for ins in ins_list:
                si = ins.get("sync_info")
                waits = si.get("on_wait") if si else None
                if waits and len(waits) > 1:
                    for i, w in enumerate(waits):
                        out.append(
                            {
                                "debug": ins.get("debug", 0),
                                "engine": ins["engine"],
                                "ins": [],
                                "outs": [],
                                "name": f"{ins['name']}-antw{i}",
                                "opcode": "EventSemaphore",
                                "sync_info": {"on_update": [], "on_wait": [w]},
                            }
                        )
                    si["on_wait"] = []
                    n_split += 1
                out.append(ins)
            blk["instructions"] = out
    return json.dumps(j).encode()


def get_program() -> "bass.Bass":
    if "nc" not in _CACHE:
        nc = build_program()
        orig = nc.to_json_bytes
        nc.to_json_bytes = lambda: _split_sync_waits_json(orig())
        _CACHE["nc"] = nc
    return _CACHE["nc"]


def prep_inputs(I, bn_gamma, bn_beta, bn_mean, bn_var, conv_w, alpha, prelu_w):
    """Host-side folding: BN -> (scale, bias); alpha -> weights; per-k lhsT
    blocks duplicated into both PE array halves."""
    f32 = np.float32
    gamma = np.asarray(bn_gamma, f32)
    beta = np.asarray(bn_beta, f32)
    mean = np.asarray(bn_mean, f32)
    var = np.asarray(bn_var, f32)
    s = gamma / np.sqrt(var + f32(BN_EPS))        # [CIN]
    t = beta - mean * s                            # [CIN]

    w = np.asarray(conv_w, f32) * np.asarray(alpha, f32)[:, None, None]  # [COUT, CIN, K]
    Wb = np.zeros((128, K * 128), np.float32)
    for k in range(K):
        Wb[0:64, 128 * k : 128 * k + 128] = w[:, :, k].T
        Wb[64:128, 128 * k : 128 * k + 128] = w[:, :, k].T
    Wb = Wb.astype(ml_dtypes.bfloat16)

    a = f32(np.asarray(prelu_w, f32).reshape(-1)[0])
    sbp = np.zeros((128, 4), f32)
    sbp[0:64, 0] = s
    sbp[64:128, 0] = s
    sbp[0:64, 1] = t
    sbp[64:128, 1] = t
    sbp[:, 2] = f32(1.0) - a
    sbp[:, 3] = a
    return Wb, sbp


def kernel(I, bn_gamma, bn_beta, bn_mean, bn_var, conv_w, alpha, prelu_w):
    I = np.ascontiguousarray(np.asarray(I, np.float32))
    assert I.shape == (B, CIN, L), I.shape
    Wb, sbp = prep_inputs(I, bn_gamma, bn_beta, bn_mean, bn_var, conv_w, alpha, prelu_w)

    nc = get_program()
    in_maps = [
        {"I8": I[BPC * c : BPC * (c + 1)], "W": Wb, "SBp": sbp} for c in range(N_CORES)
    ]
    res = run_bass_kernel_spmd(nc, in_maps, core_ids=list(range(N_CORES)))
    out = np.concatenate(
        [np.asarray(res.results[c]["O8"]) for c in range(N_CORES)], axis=0
    )
    return np.ascontiguousarray(out.astype(np.float32))

